# revision 38
# baseline (speedup 1.0000x reference)
"""Masked dot-product attention on 8 Trainium2 NeuronCores.

Problem: B=8, S=4096, D=64 fp32; per-batch key-length mask; softmax over keys.

Sharding: sequence-parallel over Q rows. Each core computes a 512-row Q slice
of all 8 batches. The key loop for batch b runs ceil(valid_len[b]/128) tiles
(same trip counts on every core -> one SPMD program, perfectly balanced
regardless of the valid_len distribution).

The kernel is one FLAT pipeline over all (batch, k-tile) pairs, chunked into
groups of 2; groups freely span batch boundaries. EVERYTHING runs in the
64x128 PE row-tiling mode -- switching tiling modes forces an array drain
(~100ns), so phase 1 AND phase 2 are built from 64-contraction matmuls that
run concurrently in the two array row-halves:
  phase 1: two psum tiles per group, ps0/ps1[k=128, q=512] = K_tile.T @ Q;
           contraction is D=64, tile t streams in array row-half (t%2)*64,
           consecutive tiles run concurrently (Q is duplicated on partitions
           64..127 to feed the upper half).  ps0 and ps1 are SEPARATE psum
           pools: each is read by exactly one exp engine -- two readers on
           one tile get chained by the dep tracker and serialize the engines.
  exp:     slot-0 tile -> one ScalarE activation, exact exp, PSUM->SBUF bf16
           (scores pre-scaled by 1/8 on the host, folded into Q; no
           max-subtraction needed, scores ~ N(0,1) +- ~8).
           slot-1 tile -> one VectorE tensor_scalar: the single-phase
           Schraudolph i16 = round(s*128*log2e + 16248.75) written via
           bitcast as the bf16 bit pattern of ~exp(s) (piecewise-linear 2^x,
           +-4% scallop, gain-calibrated unbiased; softmax mixing of exact
           even tiles and approximate odd tiles averages the error down).
           Each engine's ~660ns/tile matches the PE's ~660ns/group so exp
           never gates the pipe.  Phase-2 matmuls are emitted LAG=6 groups
           late to ride out exp latency jitter.
  phase 2: per k-tile TWO concurrent half-contraction matmuls (same 64x128
           mode, no switch): psum_a[72,512] += V[0:64].T @ E[0:64] in h0 and
           psum_b += V[64:128].T @ E[64:128] in h64 (concurrent accumulation
           into ONE bank faults on HW; two banks is exactly the phase-1
           pattern).  V is padded to 72 weight columns, col 64 = ones, so
           row 64 accumulates the softmax denominator.  The host sums the
           a/b partials.  Adjacent batches alternate accumulator pairs
           (ps_o pool bufs=4).
  tail:    ScalarE-copies psum_a[0:65] and VectorE-copies psum_b[0:65] to
           SBUF in parallel, two DMAs out. numerator/denominator divide and
           the a+b partial sum happen on the HOST.

Masking costs nothing on-device: the host zeroes V rows (incl. the ones
column) at key positions >= valid_len, so masked keys contribute 0 to both
numerator and denominator; exp of their scores is finite garbage times zero.

Perf notes: the PE-HAM clock gate keeps the array at 1.2 GHz until ~3.4us of
sustained activity -- NDUMMY warm-up matmuls on zeroed SBUF bridge the
framework preamble (~7.4us) to the first real matmul so the pipeline runs at
2.4 GHz from the start.  DMA triggers cost ~650ns each on the Sync engine,
so the first batch's K pair 0 gets its own small trigger (gates the first
matmul), and a mid-sized batch is ordered FIRST (fast ramp), largest next,
smallest last (short tail).  LDWEIGHTS hide under in-flight matmuls via the
PE's background weight buffer as long as the tiling mode never changes.
"""

import math
from contextlib import ExitStack

import numpy as np

B = 8
S = 4096
D = 64
N_CORES = 8
QB = S // N_CORES  # 512 q rows per core per batch
KT = 128  # k rows per tile
NKMAX = S // KT  # 32
NPMAX = NKMAX // 2  # 16 k-tile pairs
VC = 72  # V weight columns: 64 value dims + 1 ones col + 7 pad
GROUP = 2  # k-tiles per PSUM group / exp instruction
NDUMMY = 4  # PE warm-up matmuls during the DMA prologue (see _build_program)
WARM_COLS = 384  # columns per warm-up matmul (~430ns each at cold clock)
# Exp split: ScalarE exact-exps each group's slot-0 tile, VectorE single-op
# Schraudolph-exps the slot-1 tile -- separate destination tiles per engine
# (a shared tile serializes the engines through the dep tracker).  ScalarE
# (172+512)/1.2GHz = 570ns + both batch-end copies vs VectorE
# (120+512)/0.96GHz = 658ns: both land ~660ns, right at the PE's ~660ns
# group stream time, so exp never gates the pipeline.
SCALE = 1.0 / math.sqrt(D)  # 1/8, exact in bf16
EXP_A = 1.4426950408889634 * 128.0  # log2(e) * 2^7
EXP_B1 = 16248.75  # bf16-bits bias for the single-phase Schraudolph:
#                    16256 (bits of 1.0) - 7.25 gain calibration making the
#                    piecewise-linear 2^x approximation unbiased vs exact
#                    exp over scores ~ N(0,1) (numpy-calibrated)

_PROGRAM_CACHE: dict = {}


def _build_program(k_tiles):
    import concourse.tile as tile
    from concourse import bacc, mybir

    f32 = mybir.dt.float32
    bf16 = mybir.dt.bfloat16
    i16 = mybir.dt.int16
    nc = bacc.Bacc("TRN2", target_bir_lowering=False, debug=False,
                   enable_asserts=False, num_devices=N_CORES)

    qx = nc.dram_tensor("qx", [B, KT, QB], bf16, kind="ExternalInput").ap()
    kx = nc.dram_tensor("kx", [B, KT, NPMAX * KT], bf16,
                        kind="ExternalInput").ap()
    vx = nc.dram_tensor("vx", [B, KT, NKMAX * VC], bf16,
                        kind="ExternalInput").ap()
    # two contraction-half partials per batch; the host sums them
    out = nc.dram_tensor("out", [B, D + 1, 2 * QB], f32,
                         kind="ExternalOutput").ap()

    order = sorted(range(B), key=lambda x: -k_tiles[x])
    # ramp/tail shaping: a mid-sized batch FIRST (its K+V land quickly so the
    # pipeline is dense early), then the big batches, smallest last (short
    # tail).  The first batch's compute (~nk*0.4us) covers the next batch's
    # DMA.
    if B >= 5:
        order = [order[4]] + order[:4] + order[5:]
    flat = [(b, t) for b in order for t in range(k_tiles[b])]
    ngroups = (len(flat) + GROUP - 1) // GROUP

    with tile.TileContext(nc) as tc:
        with ExitStack() as ctx:
            q_pool = ctx.enter_context(tc.tile_pool(name="q", bufs=3))
            k_pool = ctx.enter_context(tc.tile_pool(name="k", bufs=2))
            v_pool = ctx.enter_context(tc.tile_pool(name="v", bufs=2))
            e_s_pool = ctx.enter_context(tc.tile_pool(name="es", bufs=8))
            e_v_pool = ctx.enter_context(tc.tile_pool(name="ev", bufs=8))
            o_a_pool = ctx.enter_context(tc.tile_pool(name="oa", bufs=2))
            o_b_pool = ctx.enter_context(tc.tile_pool(name="ob", bufs=2))
            warm_pool = ctx.enter_context(tc.tile_pool(name="warm", bufs=1))
            # two single-bank score tiles per group, one per exp engine --
            # a single shared score tile would chain its two readers
            # (ScalarE act -> VectorE ts) through the dep tracker and
            # serialize the exp engines
            ps_s0_pool = ctx.enter_context(
                tc.tile_pool(name="ps_s0", bufs=2, space="PSUM"))
            ps_s1_pool = ctx.enter_context(
                tc.tile_pool(name="ps_s1", bufs=2, space="PSUM"))
            ps_o_pool = ctx.enter_context(
                tc.tile_pool(name="ps_o", bufs=4, space="PSUM"))

            kt_sb = {}
            vt_sb = {}
            qt_sb = {}
            pso_a = {}
            pso_b = {}
            e_tiles = {}

            # PE-HAM warm-up: the PE clock sits gated at 1.2 GHz until the
            # activity monitor sees ~3.4us of sustained matmul traffic; the
            # real pipeline only reaches the PE at ~10us (framework preamble
            # + first DMAs), so without this the first ~11us of real matmuls
            # run at half clock.  Dummy matmuls on a zeroed SBUF tile keep
            # the PE busy from ~6us (right after the preamble) so the HAM
            # flips to 2.4 GHz before the first real matmul issues.  They
            # run in row-half h64 only, leaving h0 free so the first real
            # phase-1 LDWEIGHTS can still be pulled ahead.
            wz = warm_pool.tile([KT, 512], bf16, name="wz", tag="wz")
            nc.vector.memset(wz[:], 0)
            for _ in range(NDUMMY):
                # same tag as the real score tiles so the pool stays 2 bufs
                ps_w = ps_s1_pool.tile([KT, QB], f32, name="ps_s1")
                nc.tensor.matmul(ps_w[:, :WARM_COLS], lhsT=wz[64:, :KT],
                                 rhs=wz[64:, :WARM_COLS],
                                 start=True, stop=True)

            def load_qk(b):
                np_b = (k_tiles[b] + 1) // 2
                k_all = k_pool.tile([KT, NPMAX * KT], bf16, name=f"k{b}",
                                    tag="k")
                nc.sync.dma_start(k_all[:, :np_b * KT], kx[b][:, :np_b * KT])
                qt = q_pool.tile([KT, QB], bf16, name=f"q{b}", tag="q")
                nc.sync.dma_start(qt[:], qx[b])
                qt_sb[b], kt_sb[b] = qt, k_all

            def load_v(b):
                nk = k_tiles[b]
                v_all = v_pool.tile([KT, NKMAX * VC], bf16, name=f"v{b}",
                                    tag="v")
                nc.sync.dma_start(v_all[:, :nk * VC], vx[b][:, :nk * VC])
                vt_sb[b] = v_all

            # Prologue trigger order (all on the Sync HWDGE queue -- DMA
            # completion sems from the Scalar queue wake the PE ~3us late):
            # k(B1) pair 0 FIRST as its own small trigger (so the first
            # phase-1 matmul isn't gated on the whole K transfer), then
            # q(B1), k(B1) rest, v(B1), batch 2.
            b1, b2 = order[0], order[1]
            np1 = (k_tiles[b1] + 1) // 2
            k1 = k_pool.tile([KT, NPMAX * KT], bf16, name=f"k{b1}", tag="k")
            nc.sync.dma_start(k1[:, :KT], kx[b1][:, :KT])
            qt1 = q_pool.tile([KT, QB], bf16, name=f"q{b1}", tag="q")
            nc.sync.dma_start(qt1[:], qx[b1])
            if np1 > 1:
                nc.sync.dma_start(k1[:, KT:np1 * KT], kx[b1][:, KT:np1 * KT])
            qt_sb[b1], kt_sb[b1] = qt1, k1
            load_v(b1)
            load_qk(b2)
            load_v(b2)
            next_load = 2

            def emit_p2s(g):
                gt = flat[g * GROUP:(g + 1) * GROUP]
                e_pair = e_tiles.pop(g)
                for i, (b, t) in enumerate(gt):
                    e_sb = e_pair[i]
                    if t == 0:
                        pso_a[b] = ps_o_pool.tile([KT, QB], f32,
                                                  name=f"psoa{b}", tag="ps_o")
                        pso_b[b] = ps_o_pool.tile([KT, QB], f32,
                                                  name=f"psob{b}", tag="ps_o")
                    # phase 2 in the SAME 64x128 row-tiled mode as phase 1:
                    # the k=128 contraction splits into the two array
                    # row-halves, which stream their E halves concurrently
                    # into separate PSUM accumulators (concurrent same-bank
                    # accumulation faults on HW; separate banks is exactly
                    # the phase-1 pattern).  The host sums the two partials.
                    # No 64<->128 tiling-mode switch ever happens, which
                    # removes two array-drain stalls (~100ns each) per group.
                    last = t == k_tiles[b] - 1
                    nc.tensor.matmul(
                        pso_a[b][:VC, :],
                        lhsT=vt_sb[b][0:64, t * VC:(t + 1) * VC],
                        rhs=e_sb[0:64, :],
                        start=(t == 0), stop=last,
                        skip_group_check=True)
                    nc.tensor.matmul(
                        pso_b[b][:VC, :],
                        lhsT=vt_sb[b][64:128, t * VC:(t + 1) * VC],
                        rhs=e_sb[64:128, :],
                        start=(t == 0), stop=last,
                        skip_group_check=True)
                    if last:
                        # both partial-copies on ScalarE: routing one to
                        # VectorE stalls the V exp pipeline at batch
                        # boundaries (measured ~+12us total)
                        o_na = o_a_pool.tile([D + 1, QB], f32, name=f"oa{b}",
                                             tag="o_a")
                        nc.scalar.copy(o_na[:], pso_a[b][:D + 1, :])
                        nc.sync.dma_start(out[b][:, :QB], o_na[:])
                        o_nb = o_b_pool.tile([D + 1, QB], f32, name=f"ob{b}",
                                             tag="o_b")
                        nc.scalar.copy(o_nb[:], pso_b[b][:D + 1, :])
                        nc.sync.dma_start(out[b][:, QB:], o_nb[:])

            LAG = 6  # P2(g) is emitted at iteration g+LAG: the exp engines
            #          get ~LAG group-periods of latency slack, and the PE
            #          queue never stalls on an in-flight exp (traces showed
            #          p2 matmuls waiting on the 3-op DVE exp chain at LAG=4).
            def emit_p1s(g):
                gt = flat[g * GROUP:(g + 1) * GROUP]
                # prefetch the next batch when a new batch first appears
                for (b, t) in gt:
                    if t == 0 and b != order[0] and next_load[0] < B:
                        load_qk(order[next_load[0]])
                        load_v(order[next_load[0]])
                        next_load[0] += 1
                ps0 = ps_s0_pool.tile([KT, QB], f32, name="ps_s0")
                ps1 = None
                if len(gt) > 1:
                    ps1 = ps_s1_pool.tile([KT, QB], f32, name="ps_s1")
                for i, (b, t) in enumerate(gt):
                    p, half = divmod(t, 2)
                    lo = 64 * half
                    nc.tensor.matmul(
                        (ps0 if i == 0 else ps1)[:, :],
                        lhsT=kt_sb[b][lo:lo + 64, p * KT:(p + 1) * KT],
                        rhs=qt_sb[b][lo:lo + 64, :],
                        start=True, stop=True)
                return ps0, ps1

            def emit_exp(g, pss):
                ps0, ps1 = pss
                # slot 0 -> ScalarE exact exp; slot 1 -> VectorE single-op
                # Schraudolph: one tensor_scalar whose i16 result IS the
                # bf16 bit pattern of ~exp(s), written via bitcast.
                e_s = e_s_pool.tile([KT, QB], bf16, name="e_s")
                nc.scalar.activation(
                    e_s[:], ps0[:, :],
                    mybir.ActivationFunctionType.Exp)
                e_v = None
                if ps1 is not None:
                    e_v = e_v_pool.tile([KT, QB], bf16, name="e_v")
                    nc.vector.tensor_scalar(
                        e_v[:].bitcast(i16), ps1[:, :],
                        EXP_A, EXP_B1,
                        mybir.AluOpType.mult, mybir.AluOpType.add)
                e_tiles[g] = (e_s, e_v)

            # two groups per iteration: P1s of both, then both exps, then
            # both lagged P2 blocks -- halves the PE's 64x128 <-> 128x128
            # weight-mode switches.
            next_load = [next_load]
            for g0 in range(0, ngroups, 2):
                gs = [g for g in (g0, g0 + 1) if g < ngroups]
                pss = [emit_p1s(g) for g in gs]
                for g, ps_s in zip(gs, pss):
                    emit_exp(g, ps_s)
                for g in gs:
                    if g >= LAG:
                        emit_p2s(g - LAG)
            for g in range(max(0, ngroups - LAG), ngroups):
                emit_p2s(g)

    nc.compile()
    return nc


def _prep_inputs(query, key, value, valid):
    import ml_dtypes

    vclamp = np.clip(valid, 1, S)
    k_tiles = tuple(int(x) for x in np.ceil(vclamp / KT).astype(np.int64))

    # K packed for 64x128 row-tiling: pair p holds k-tile 2p on partitions
    # 0..63 and k-tile 2p+1 on partitions 64..127, at columns [128p, 128p+128).
    kt4 = key.reshape(B, NPMAX, 2, KT, D)  # [B, pair, half, key, d]
    kxh = np.ascontiguousarray(
        kt4.transpose(0, 2, 4, 1, 3).reshape(B, KT, NPMAX * KT)
    ).astype(ml_dtypes.bfloat16)

    vxh = np.zeros((B, S, VC), dtype=np.float32)  # padded to 72 weight cols
    vxh[:, :, :D] = value
    vxh[:, :, D] = 1.0
    for b in range(B):
        vxh[b, vclamp[b]:, :] = 0.0  # masked keys contribute nothing
    # [B, S, 72] -> [B, KT, NKMAX*72]: per-partition contiguous k-tile runs
    vxt = np.ascontiguousarray(
        vxh.reshape(B, NKMAX, KT, VC).transpose(0, 2, 1, 3).reshape(
            B, KT, NKMAX * VC)
    ).astype(ml_dtypes.bfloat16)

    # Q scaled by 1/sqrt(D) (exact power of two) and duplicated onto
    # partitions 64..127 to feed the upper row-half of the PE array.
    qs = (query * SCALE).transpose(0, 2, 1)  # [B, D, S]

    in_maps = []
    for c in range(N_CORES):
        qc = qs[:, :, c * QB:(c + 1) * QB]  # [B, D, QB]
        qxh = np.concatenate([qc, qc], axis=1).astype(ml_dtypes.bfloat16)
        in_maps.append({"qx": np.ascontiguousarray(qxh),
                        "kx": kxh, "vx": vxt})
    return k_tiles, in_maps


def kernel(query, key, value, valid_len):
    from concourse.bass_utils import run_bass_kernel_spmd

    query = np.ascontiguousarray(query, dtype=np.float32)
    key = np.ascontiguousarray(key, dtype=np.float32)
    value = np.ascontiguousarray(value, dtype=np.float32)
    valid = np.asarray(valid_len).astype(np.int64)
    assert query.shape == (B, S, D) and key.shape == (B, S, D)
    assert value.shape == (B, S, D) and valid.shape == (B,)

    k_tiles, in_maps = _prep_inputs(query, key, value, valid)

    nc = _PROGRAM_CACHE.get(k_tiles)
    if nc is None:
        nc = _build_program(k_tiles)
        _PROGRAM_CACHE[k_tiles] = nc

    res = run_bass_kernel_spmd(nc, in_maps, core_ids=list(range(N_CORES)))

    full = np.empty((B, S, D), dtype=np.float32)
    for c in range(N_CORES):
        # [B, 65, 2*QB]: two contraction-half partials (numerator rows +
        # denominator row); sum the halves, then divide.
        o = res.results[c]["out"]
        oh = o[:, :, :QB] + o[:, :, QB:]
        full[:, c * QB:(c + 1) * QB, :] = (
            oh[:, :D, :] / oh[:, D:D + 1, :]).transpose(0, 2, 1)

    # valid_len == 0 never occurs per the spec (randint >= 1), but the
    # reference would produce uniform attention there; match it exactly.
    if np.any(valid < 1):
        for b in np.nonzero(valid < 1)[0]:
            sc = (query[b] @ key[b].T) * SCALE - 1.0e6
            a = np.exp(sc - sc.max(axis=-1, keepdims=True))
            a /= a.sum(axis=-1, keepdims=True)
            full[b] = a @ value[b]

    return full



# revision 39
# speedup vs baseline: 1.1771x; 1.1771x over previous
"""Masked dot-product attention on 8 Trainium2 NeuronCores.

Problem: B=8, S=4096, D=64 fp32; per-batch key-length mask; softmax over keys.

Sharding: sequence-parallel over Q rows. Each core computes a 512-row Q slice
of all 8 batches. The key loop for batch b runs ceil(valid_len[b]/128) tiles
(same trip counts on every core -> one SPMD program, perfectly balanced
regardless of the valid_len distribution).

The kernel is one FLAT pipeline over all (batch, k-tile) pairs, chunked into
groups of 2; groups freely span batch boundaries. EVERYTHING runs in the
64x128 PE row-tiling mode -- switching tiling modes forces an array drain
(~100ns), so phase 1 AND phase 2 are built from 64-contraction matmuls that
run concurrently in the two array row-halves:
  phase 1: two psum tiles per group, ps0/ps1[k=128, q=512] = K_tile.T @ Q;
           contraction is D=64, tile t streams in array row-half (t%2)*64,
           consecutive tiles run concurrently (Q is duplicated on partitions
           64..127 to feed the upper half).  ps0 and ps1 are SEPARATE psum
           pools: each is read by exactly one exp engine -- two readers on
           one tile get chained by the dep tracker and serialize the engines.
  exp:     slot-0 tile -> one ScalarE activation, exact exp, PSUM->SBUF bf16
           (scores pre-scaled by 1/8 on the host, folded into Q; no
           max-subtraction needed, scores ~ N(0,1) +- ~8).
           slot-1 tile -> one VectorE tensor_scalar: the single-phase
           Schraudolph i16 = round(s*128*log2e + 16248.75) written via
           bitcast as the bf16 bit pattern of ~exp(s) (piecewise-linear 2^x,
           +-4% scallop, gain-calibrated unbiased; softmax mixing of exact
           even tiles and approximate odd tiles averages the error down).
           Each engine's ~660ns/tile matches the PE's ~660ns/group so exp
           never gates the pipe.  Phase-2 matmuls are emitted LAG=6 groups
           late to ride out exp latency jitter.
  phase 2: per k-tile TWO concurrent half-contraction matmuls (same 64x128
           mode, no switch): psum_a[72,512] += V[0:64].T @ E[0:64] in h0 and
           psum_b += V[64:128].T @ E[64:128] in h64 (concurrent accumulation
           into ONE bank faults on HW; two banks is exactly the phase-1
           pattern).  V is padded to 72 weight columns, col 64 = ones, so
           row 64 accumulates the softmax denominator.  The host sums the
           a/b partials.  Adjacent batches alternate accumulator pairs
           (ps_o pool bufs=4).
  tail:    ScalarE-copies psum_a[0:65] and VectorE-copies psum_b[0:65] to
           SBUF in parallel, two DMAs out. numerator/denominator divide and
           the a+b partial sum happen on the HOST.

Masking costs nothing on-device: the host zeroes V rows (incl. the ones
column) at key positions >= valid_len, so masked keys contribute 0 to both
numerator and denominator; exp of their scores is finite garbage times zero.

Perf notes: the PE-HAM clock gate keeps the array at 1.2 GHz until ~3.4us of
sustained activity -- NDUMMY warm-up matmuls on zeroed SBUF bridge the
framework preamble (~7.4us) to the first real matmul so the pipeline runs at
2.4 GHz from the start.  DMA triggers cost ~650ns each on the Sync engine,
so the first batch's K pair 0 gets its own small trigger (gates the first
matmul), and a mid-sized batch is ordered FIRST (fast ramp), largest next,
smallest last (short tail).  LDWEIGHTS hide under in-flight matmuls via the
PE's background weight buffer as long as the tiling mode never changes.
"""

import math
from contextlib import ExitStack

import numpy as np

B = 8
S = 4096
D = 64
N_CORES = 8
QB = S // N_CORES  # 512 q rows per core per batch
KT = 128  # k rows per tile
NKMAX = S // KT  # 32
NPMAX = NKMAX // 2  # 16 k-tile pairs
VC = 72  # V weight columns: 64 value dims + 1 ones col + 7 pad
GROUP = 2  # k-tiles per PSUM group / exp instruction
NDUMMY = 10  # PE warm-up matmuls: the dummy stream alone must span a full
#              ~3.4us HAM activity window (the early real pipeline is too
#              thin, DMA-paced, to flip the clock gate by itself)
WARM_COLS = 384  # columns per warm-up matmul (~430ns each at cold clock)
# Exp split: ScalarE exact-exps each group's slot-0 tile, VectorE single-op
# Schraudolph-exps the slot-1 tile -- separate destination tiles per engine
# (a shared tile serializes the engines through the dep tracker).  ScalarE
# (172+512)/1.2GHz = 570ns + both batch-end copies vs VectorE
# (120+512)/0.96GHz = 658ns: both land ~660ns, right at the PE's ~660ns
# group stream time, so exp never gates the pipeline.
SCALE = 1.0 / math.sqrt(D)  # 1/8, exact in bf16
EXP_A = 1.4426950408889634 * 128.0  # log2(e) * 2^7
EXP_B1 = 16248.75  # bf16-bits bias for the single-phase Schraudolph:
#                    16256 (bits of 1.0) - 7.25 gain calibration making the
#                    piecewise-linear 2^x approximation unbiased vs exact
#                    exp over scores ~ N(0,1) (numpy-calibrated)

_PROGRAM_CACHE: dict = {}


def _build_program(k_tiles):
    import concourse.tile as tile
    from concourse import bacc, mybir

    f32 = mybir.dt.float32
    bf16 = mybir.dt.bfloat16
    i16 = mybir.dt.int16
    nc = bacc.Bacc("TRN2", target_bir_lowering=False, debug=False,
                   enable_asserts=False, num_devices=N_CORES)

    qx = nc.dram_tensor("qx", [B, KT, QB], bf16, kind="ExternalInput").ap()
    kx = nc.dram_tensor("kx", [B, KT, NPMAX * KT], bf16,
                        kind="ExternalInput").ap()
    vx = nc.dram_tensor("vx", [B, KT, NKMAX * VC], bf16,
                        kind="ExternalInput").ap()
    # two contraction-half partials per batch; the host sums them
    out = nc.dram_tensor("out", [B, D + 1, 2 * QB], f32,
                         kind="ExternalOutput").ap()

    order = sorted(range(B), key=lambda x: -k_tiles[x])
    # ramp/tail shaping: a mid-sized batch FIRST (its K+V land quickly so the
    # pipeline is dense early), then the big batches, smallest last (short
    # tail).  The first batch's compute (~nk*0.4us) covers the next batch's
    # DMA.
    if B >= 5:
        order = [order[4]] + order[:4] + order[5:]
    flat = [(b, t) for b in order for t in range(k_tiles[b])]
    ngroups = (len(flat) + GROUP - 1) // GROUP

    with tile.TileContext(nc) as tc:
        with ExitStack() as ctx:
            q_pool = ctx.enter_context(tc.tile_pool(name="q", bufs=3))
            k_pool = ctx.enter_context(tc.tile_pool(name="k", bufs=2))
            v_pool = ctx.enter_context(tc.tile_pool(name="v", bufs=2))
            e_s_pool = ctx.enter_context(tc.tile_pool(name="es", bufs=8))
            e_v_pool = ctx.enter_context(tc.tile_pool(name="ev", bufs=8))
            o_a_pool = ctx.enter_context(tc.tile_pool(name="oa", bufs=2))
            o_b_pool = ctx.enter_context(tc.tile_pool(name="ob", bufs=2))
            warm_pool = ctx.enter_context(tc.tile_pool(name="warm", bufs=1))
            # two single-bank score tiles per group, one per exp engine --
            # a single shared score tile would chain its two readers
            # (ScalarE act -> VectorE ts) through the dep tracker and
            # serialize the exp engines
            ps_s0_pool = ctx.enter_context(
                tc.tile_pool(name="ps_s0", bufs=2, space="PSUM"))
            ps_s1_pool = ctx.enter_context(
                tc.tile_pool(name="ps_s1", bufs=2, space="PSUM"))
            ps_o_pool = ctx.enter_context(
                tc.tile_pool(name="ps_o", bufs=4, space="PSUM"))

            kt_sb = {}
            vt_sb = {}
            qt_sb = {}
            pso_a = {}
            pso_b = {}
            e_tiles = {}

            # PE-HAM warm-up: the PE clock sits gated at 1.2 GHz until the
            # activity monitor sees ~3.4us of sustained matmul traffic; the
            # real pipeline only reaches the PE at ~10us (framework preamble
            # + first DMAs), so without this the first ~11us of real matmuls
            # run at half clock.  Dummy matmuls on a zeroed SBUF tile keep
            # the PE busy from ~6us (right after the preamble) so the HAM
            # flips to 2.4 GHz before the first real matmul issues.  They
            # run in row-half h64 only, leaving h0 free so the first real
            # phase-1 LDWEIGHTS can still be pulled ahead.
            wz = warm_pool.tile([KT, 512], bf16, name="wz", tag="wz")
            nc.vector.memset(wz[:], 0)
            for _ in range(NDUMMY):
                # same tag as the real score tiles so the pool stays 2 bufs
                ps_w = ps_s1_pool.tile([KT, QB], f32, name="ps_s1")
                nc.tensor.matmul(ps_w[:, :WARM_COLS], lhsT=wz[64:, :KT],
                                 rhs=wz[64:, :WARM_COLS],
                                 start=True, stop=True)

            def load_qk(b):
                np_b = (k_tiles[b] + 1) // 2
                k_all = k_pool.tile([KT, NPMAX * KT], bf16, name=f"k{b}",
                                    tag="k")
                nc.sync.dma_start(k_all[:, :np_b * KT], kx[b][:, :np_b * KT])
                qt = q_pool.tile([KT, QB], bf16, name=f"q{b}", tag="q")
                nc.sync.dma_start(qt[:], qx[b])
                qt_sb[b], kt_sb[b] = qt, k_all

            def load_v(b):
                nk = k_tiles[b]
                v_all = v_pool.tile([KT, NKMAX * VC], bf16, name=f"v{b}",
                                    tag="v")
                nc.sync.dma_start(v_all[:, :nk * VC], vx[b][:, :nk * VC])
                vt_sb[b] = v_all

            # Prologue trigger order (all on the Sync HWDGE queue -- DMA
            # completion sems from the Scalar queue wake the PE ~3us late):
            # k(B1) pair 0 FIRST as its own small trigger (so the first
            # phase-1 matmul isn't gated on the whole K transfer), then
            # q(B1), k(B1) rest, v(B1), batch 2.
            b1, b2 = order[0], order[1]
            np1 = (k_tiles[b1] + 1) // 2
            k1 = k_pool.tile([KT, NPMAX * KT], bf16, name=f"k{b1}", tag="k")
            nc.sync.dma_start(k1[:, :KT], kx[b1][:, :KT])
            qt1 = q_pool.tile([KT, QB], bf16, name=f"q{b1}", tag="q")
            nc.sync.dma_start(qt1[:], qx[b1])
            if np1 > 1:
                nc.sync.dma_start(k1[:, KT:np1 * KT], kx[b1][:, KT:np1 * KT])
            qt_sb[b1], kt_sb[b1] = qt1, k1
            load_v(b1)
            load_qk(b2)
            load_v(b2)
            next_load = 2

            def emit_p2s(g):
                gt = flat[g * GROUP:(g + 1) * GROUP]
                e_pair = e_tiles.pop(g)
                for i, (b, t) in enumerate(gt):
                    e_sb = e_pair[i]
                    if t == 0:
                        pso_a[b] = ps_o_pool.tile([KT, QB], f32,
                                                  name=f"psoa{b}", tag="ps_o")
                        pso_b[b] = ps_o_pool.tile([KT, QB], f32,
                                                  name=f"psob{b}", tag="ps_o")
                    # phase 2 in the SAME 64x128 row-tiled mode as phase 1:
                    # the k=128 contraction splits into the two array
                    # row-halves, which stream their E halves concurrently
                    # into separate PSUM accumulators (concurrent same-bank
                    # accumulation faults on HW; separate banks is exactly
                    # the phase-1 pattern).  The host sums the two partials.
                    # No 64<->128 tiling-mode switch ever happens, which
                    # removes two array-drain stalls (~100ns each) per group.
                    last = t == k_tiles[b] - 1
                    nc.tensor.matmul(
                        pso_a[b][:VC, :],
                        lhsT=vt_sb[b][0:64, t * VC:(t + 1) * VC],
                        rhs=e_sb[0:64, :],
                        start=(t == 0), stop=last,
                        skip_group_check=True)
                    nc.tensor.matmul(
                        pso_b[b][:VC, :],
                        lhsT=vt_sb[b][64:128, t * VC:(t + 1) * VC],
                        rhs=e_sb[64:128, :],
                        start=(t == 0), stop=last,
                        skip_group_check=True)
                    if last:
                        # both partial-copies on ScalarE: routing one to
                        # VectorE stalls the V exp pipeline at batch
                        # boundaries (measured ~+12us total)
                        o_na = o_a_pool.tile([D + 1, QB], f32, name=f"oa{b}",
                                             tag="o_a")
                        nc.scalar.copy(o_na[:], pso_a[b][:D + 1, :])
                        nc.sync.dma_start(out[b][:, :QB], o_na[:])
                        o_nb = o_b_pool.tile([D + 1, QB], f32, name=f"ob{b}",
                                             tag="o_b")
                        nc.scalar.copy(o_nb[:], pso_b[b][:D + 1, :])
                        nc.sync.dma_start(out[b][:, QB:], o_nb[:])

            LAG = 6  # P2(g) is emitted at iteration g+LAG: the exp engines
            #          get ~LAG group-periods of latency slack, and the PE
            #          queue never stalls on an in-flight exp (traces showed
            #          p2 matmuls waiting on the 3-op DVE exp chain at LAG=4).
            def emit_p1s(g):
                gt = flat[g * GROUP:(g + 1) * GROUP]
                # prefetch the next batch when a new batch first appears
                for (b, t) in gt:
                    if t == 0 and b != order[0] and next_load[0] < B:
                        load_qk(order[next_load[0]])
                        load_v(order[next_load[0]])
                        next_load[0] += 1
                ps0 = ps_s0_pool.tile([KT, QB], f32, name="ps_s0")
                ps1 = None
                if len(gt) > 1:
                    ps1 = ps_s1_pool.tile([KT, QB], f32, name="ps_s1")
                for i, (b, t) in enumerate(gt):
                    p, half = divmod(t, 2)
                    lo = 64 * half
                    nc.tensor.matmul(
                        (ps0 if i == 0 else ps1)[:, :],
                        lhsT=kt_sb[b][lo:lo + 64, p * KT:(p + 1) * KT],
                        rhs=qt_sb[b][lo:lo + 64, :],
                        start=True, stop=True)
                return ps0, ps1

            def emit_exp(g, pss):
                ps0, ps1 = pss
                # slot 0 -> ScalarE exact exp; slot 1 -> VectorE single-op
                # Schraudolph: one tensor_scalar whose i16 result IS the
                # bf16 bit pattern of ~exp(s), written via bitcast.
                e_s = e_s_pool.tile([KT, QB], bf16, name="e_s")
                nc.scalar.activation(
                    e_s[:], ps0[:, :],
                    mybir.ActivationFunctionType.Exp)
                e_v = None
                if ps1 is not None:
                    e_v = e_v_pool.tile([KT, QB], bf16, name="e_v")
                    nc.vector.tensor_scalar(
                        e_v[:].bitcast(i16), ps1[:, :],
                        EXP_A, EXP_B1,
                        mybir.AluOpType.mult, mybir.AluOpType.add)
                e_tiles[g] = (e_s, e_v)

            # two groups per iteration: P1s of both, then both exps, then
            # both lagged P2 blocks -- halves the PE's 64x128 <-> 128x128
            # weight-mode switches.
            next_load = [next_load]
            for g0 in range(0, ngroups, 2):
                gs = [g for g in (g0, g0 + 1) if g < ngroups]
                pss = [emit_p1s(g) for g in gs]
                for g, ps_s in zip(gs, pss):
                    emit_exp(g, ps_s)
                for g in gs:
                    if g >= LAG:
                        emit_p2s(g - LAG)
            for g in range(max(0, ngroups - LAG), ngroups):
                emit_p2s(g)

    nc.compile()
    return nc


def _prep_inputs(query, key, value, valid):
    import ml_dtypes

    vclamp = np.clip(valid, 1, S)
    k_tiles = tuple(int(x) for x in np.ceil(vclamp / KT).astype(np.int64))

    # K packed for 64x128 row-tiling: pair p holds k-tile 2p on partitions
    # 0..63 and k-tile 2p+1 on partitions 64..127, at columns [128p, 128p+128).
    kt4 = key.reshape(B, NPMAX, 2, KT, D)  # [B, pair, half, key, d]
    kxh = np.ascontiguousarray(
        kt4.transpose(0, 2, 4, 1, 3).reshape(B, KT, NPMAX * KT)
    ).astype(ml_dtypes.bfloat16)

    vxh = np.zeros((B, S, VC), dtype=np.float32)  # padded to 72 weight cols
    vxh[:, :, :D] = value
    vxh[:, :, D] = 1.0
    for b in range(B):
        vxh[b, vclamp[b]:, :] = 0.0  # masked keys contribute nothing
    # [B, S, 72] -> [B, KT, NKMAX*72]: per-partition contiguous k-tile runs
    vxt = np.ascontiguousarray(
        vxh.reshape(B, NKMAX, KT, VC).transpose(0, 2, 1, 3).reshape(
            B, KT, NKMAX * VC)
    ).astype(ml_dtypes.bfloat16)

    # Q scaled by 1/sqrt(D) (exact power of two) and duplicated onto
    # partitions 64..127 to feed the upper row-half of the PE array.
    qs = (query * SCALE).transpose(0, 2, 1)  # [B, D, S]

    in_maps = []
    for c in range(N_CORES):
        qc = qs[:, :, c * QB:(c + 1) * QB]  # [B, D, QB]
        qxh = np.concatenate([qc, qc], axis=1).astype(ml_dtypes.bfloat16)
        in_maps.append({"qx": np.ascontiguousarray(qxh),
                        "kx": kxh, "vx": vxt})
    return k_tiles, in_maps


def kernel(query, key, value, valid_len):
    from concourse.bass_utils import run_bass_kernel_spmd

    query = np.ascontiguousarray(query, dtype=np.float32)
    key = np.ascontiguousarray(key, dtype=np.float32)
    value = np.ascontiguousarray(value, dtype=np.float32)
    valid = np.asarray(valid_len).astype(np.int64)
    assert query.shape == (B, S, D) and key.shape == (B, S, D)
    assert value.shape == (B, S, D) and valid.shape == (B,)

    k_tiles, in_maps = _prep_inputs(query, key, value, valid)

    nc = _PROGRAM_CACHE.get(k_tiles)
    if nc is None:
        nc = _build_program(k_tiles)
        _PROGRAM_CACHE[k_tiles] = nc

    res = run_bass_kernel_spmd(nc, in_maps, core_ids=list(range(N_CORES)))

    full = np.empty((B, S, D), dtype=np.float32)
    for c in range(N_CORES):
        # [B, 65, 2*QB]: two contraction-half partials (numerator rows +
        # denominator row); sum the halves, then divide.
        o = res.results[c]["out"]
        oh = o[:, :, :QB] + o[:, :, QB:]
        full[:, c * QB:(c + 1) * QB, :] = (
            oh[:, :D, :] / oh[:, D:D + 1, :]).transpose(0, 2, 1)

    # valid_len == 0 never occurs per the spec (randint >= 1), but the
    # reference would produce uniform attention there; match it exactly.
    if np.any(valid < 1):
        for b in np.nonzero(valid < 1)[0]:
            sc = (query[b] @ key[b].T) * SCALE - 1.0e6
            a = np.exp(sc - sc.max(axis=-1, keepdims=True))
            a /= a.sum(axis=-1, keepdims=True)
            full[b] = a @ value[b]

    return full

